# revision 26
# baseline (speedup 1.0000x reference)
"""Trainium2 Bass kernel for nn_AdaptiveAttention (sparse attention, B=4 S=1024 HID=1024 H=16).

Sharding (8 cores): core c = (batch b=c//2) x (head-group g=c%2, 8 heads / 512 hid cols).

Per-core pipeline (all matmuls bf16, fp32 PSUM accumulation):
- Host prep is layout-only: per-core slices, x[b] transposed to x^T [hid, s],
  mask slice pre-transposed to [h, k, q] as bf16 0/1, Wq/Wk/Wv sliced by
  column group, Wo sliced by ROW group (row-parallel out projection).
- Q^T/K^T = W (stationary) x x^T (moving); temperature/sqrt(D) folded into the
  Q eviction scale+bias. V computed in native [s, cols] layout with an
  appended ones-column per head.
- Scores computed transposed [k, q] per head with head-PAIR packing on the PE
  array via tile_position (0,0)/(64,0), in half-width (512-q) windows so
  PSUM banks stay free for interleaved projection matmuls: the PE stream mixes
  score/AV matmuls with the next head-pair's Q/K projection (and V / out-proj
  chunks) so the tensor engine never idles while ACT runs exp.
- exp on ACT directly from PSUM (no max-subtraction: scores are bounded << 88,
  softmax is shift-invariant); mask applied as a single bf16 DVE multiply.
- AV = Vext^T @ P^T accumulated per k-tile; the ones column makes PSUM row 64
  the softmax denominators for free. Normalization = reciprocal + DMA
  partition-broadcast fused into the eviction multiply.
- Out projection is ROW-parallel: partial_out[q, :] = att_localT^T @ Wo[rows]
  computed entirely on-core (no collectives); the host sums the two partials
  of each batch during unshard and adds the (bv @ Wo + bo) bias row there
  (softmax rows sum to 1, so bv contributes a constant row).
- DMAs are consolidated (one per weight matrix / x / mask head) to amortize
  descriptor-generation overhead.
"""
import os
import sys

for _p in ("/opt/trn_rl_repo", "/root/.axon_site/_ro/trn_rl_repo"):
    if os.path.isdir(_p) and _p not in sys.path:
        sys.path.insert(0, _p)

import numpy as np
import ml_dtypes

import concourse.bass as bass
from concourse import bacc
import concourse.mybir as mybir
import concourse.tile as tile
from concourse.bass_utils import run_bass_kernel_spmd

B, S, HID, H, D = 4, 1024, 1024, 16, 64
NCORES = 8
GH = 8          # heads per core
LOC = GH * D    # 512, local hid slice
CORE_IDS = list(range(NCORES))

bf16 = mybir.dt.bfloat16
f32 = mybir.dt.float32
AF = mybir.ActivationFunctionType
ALU = mybir.AluOpType

_NC_CACHE = None


def _build(dbg=False, reps=1):
    nc = bacc.Bacc("TRN2", debug=False, num_devices=NCORES)

    xT = nc.declare_dram_parameter("xT", [HID, S], bf16, False)
    wq = nc.declare_dram_parameter("wq", [HID, LOC], bf16, False)
    wk = nc.declare_dram_parameter("wk", [HID, LOC], bf16, False)
    wv = nc.declare_dram_parameter("wv", [HID, LOC], bf16, False)
    wo = nc.declare_dram_parameter("wo", [LOC, HID], bf16, False)  # row slice
    maskT = nc.declare_dram_parameter("maskT", [GH, S, S], bf16, False)
    tempx = nc.declare_dram_parameter("tempx", [LOC], f32, False)  # temp[h]/sqrt(D) per col
    bqv = nc.declare_dram_parameter("bqv", [LOC], f32, False)
    bkv = nc.declare_dram_parameter("bkv", [LOC], f32, False)
    out = nc.declare_dram_parameter("out", [S, HID], f32, True)    # partial

    with tile.TileContext(nc) as tc:
        with (
            tc.tile_pool(name="pw", bufs=4) as pw,           # weights [128,4096] bf16
            tc.tile_pool(name="pxt", bufs=1) as pxt,         # xT [128,8192] bf16
            tc.tile_pool(name="pqk", bufs=8) as pqk,         # QT/KT [128,1024] bf16
            tc.tile_pool(name="pv", bufs=8) as pv,           # Vext [128,520] bf16
            tc.tile_pool(name="ppt", bufs=10) as ppt,        # P^T halves [128,512] bf16
            tc.tile_pool(name="pmask", bufs=4) as pmask,     # mask head [128,8192] bf16
            tc.tile_pool(name="pattl", bufs=4) as pattl,     # local attT bf16 (live to outproj)
            tc.tile_pool(name="pout", bufs=4) as pout,       # out staging f32
            tc.tile_pool(name="prb", bufs=4) as prb,         # recip bcast [128,512] f32
            tc.tile_pool(name="pdm", bufs=2) as pdm,         # recip [1,1024] f32
            tc.tile_pool(name="pconst", bufs=1) as pconst,   # small tiles
            tc.tile_pool(name="psc", bufs=2, space="PSUM") as psc,   # score halves [128,512]
            tc.tile_pool(name="pqs", bufs=1, space="PSUM") as pqs,   # qkv proj [128,512]
            tc.tile_pool(name="pav", bufs=4, space="PSUM") as pav,   # av [65,512] x2
            tc.tile_pool(name="pos", bufs=1, space="PSUM") as pos,   # outproj [128,512]
            tc.tile_pool(name="pdram", bufs=4, space="DRAM") as pdram,
        ):
            for _rep in range(reps):
                # ---- small constants ----
                def load_small(name, dram, cshape, rearr=None, tag=None):
                    t = pconst.tile(cshape, f32, tag=tag or name, name=name)
                    src = dram[:]
                    if rearr is not None:
                        src = src.rearrange(rearr, p=cshape[0])
                    nc.gpsimd.dma_start(out=t[:], in_=src)
                    return t

                scale_t = load_small("scale", tempx, [128, 4], "(c p) -> p c")
                bq_t = load_small("bq", bqv, [128, 4], "(c p) -> p c")
                bk_t = load_small("bk", bkv, [128, 4], "(c p) -> p c")

                # bq * scale (fold temperature/sqrt(D) into Q bias)
                bqs_t = pconst.tile([128, 4], f32, tag="bqs")
                nc.vector.tensor_mul(bqs_t[:], bq_t[:], scale_t[:])
                # pre-load the exp activation table during the DMA phase
                warm_t = pconst.tile([1, 4], f32, tag="warm")
                nc.scalar.activation(warm_t[:], scale_t[0:1, :], AF.Exp)

                # ---- bulk loads: j=0 slices of Wq/Wk land first so the
                # first window starts as early as possible ----
                wqb = pw.tile([128, 8 * 512], bf16, tag="w", name="wq")
                wqv = wqb[:].rearrange("p (c n) -> p c n", c=8)
                nc.sync.dma_start(out=wqv[:, :, 0:128],
                                  in_=wq[:, 0:128].rearrange("(c p) n -> p c n", p=128))
                xt = pxt.tile([128, 8 * 1024], bf16, tag="xt", name="xt")
                xtv = xt[:].rearrange("p (c s) -> p c s", c=8)
                for xq in range(4):
                    nc.sync.dma_start(
                        out=xtv[:, 2 * xq:2 * xq + 2, :],
                        in_=xT[256 * xq:256 * (xq + 1), :].rearrange(
                            "(c p) s -> p c s", p=128))
                wkb = pw.tile([128, 8 * 512], bf16, tag="w", name="wk")
                wkv = wkb[:].rearrange("p (c n) -> p c n", c=8)
                nc.gpsimd.dma_start(out=wkv[:, :, 0:128],
                                  in_=wk[:, 0:128].rearrange("(c p) n -> p c n", p=128))
                wvb = pw.tile([128, 8 * 512], bf16, tag="w", name="wv")
                nc.gpsimd.dma_start(out=wvb[:].rearrange("p (c n) -> p c n", c=8),
                                  in_=wv[:].rearrange("(c p) n -> p c n", p=128))
                wob = pw.tile([128, 4 * 1024], bf16, tag="w", name="wo")

                mh = [None] * GH

                xt3 = xt[:].rearrange("p (c s) -> p c s", c=8)
                wq3 = wqb[:].rearrange("p (c n) -> p c n", c=8)
                wk3 = wkb[:].rearrange("p (c n) -> p c n", c=8)
                wv3 = wvb[:].rearrange("p (c n) -> p c n", c=8)
                wo3 = wob[:].rearrange("p (r n) -> p r n", r=4)

                # ---- mask loads: one consolidated DMA per head, pool-throttled ----
                def load_mask(h):
                    t = pmask.tile([128, 8 * 1024], bf16, tag="mask", name=f"mh{h}")
                    tv = t[:].rearrange("p (k q) -> p k q", k=8)
                    for qr in range(4):
                        nc.gpsimd.dma_start(
                            out=tv[:, 2 * qr:2 * qr + 2, :],
                            in_=maskT[h, 256 * qr:256 * (qr + 1), :]
                            .rearrange("(k p) q -> p k q", p=128))
                    mh[h] = t

                def load_mask_q(h):
                    t = pmask.tile([128, 8 * 1024], bf16, tag="mask", name=f"mh{h}")
                    mh[h] = t

                mh01_views = {}
                for h in (0, 1):
                    load_mask_q(h)
                    mh01_views[h] = mh[h][:].rearrange("p (k q) -> p k q", k=8)
                for quarter in range(4):
                    for h in (0, 1):
                        nc.gpsimd.dma_start(
                            out=mh01_views[h][:, 2 * quarter:2 * quarter + 2, :],
                            in_=maskT[h, 256 * quarter:256 * (quarter + 1), :]
                            .rearrange("(k p) q -> p k q", p=128))
                    if quarter == 2:
                        nc.sync.dma_start(
                            out=wqv[:, :, 128:512],
                            in_=wq[:, 128:512].rearrange("(c p) n -> p c n", p=128))
                    elif quarter == 3:
                        nc.sync.dma_start(
                            out=wkv[:, :, 128:512],
                            in_=wk[:, 128:512].rearrange("(c p) n -> p c n", p=128))


                # ---- V projection chunk st -> Vext [128 s, 8*65] with ones col ----
                vext = [None] * 8

                def vchunk(st):
                    pool_, ptag = (pqs, "qs") if st % 2 == 0 else (pos, "os")
                    vps = pool_.tile([128, 512], f32, tag=ptag, name=f"vps{st}")
                    for c8 in range(8):
                        nc.tensor.matmul(vps[:], xt3[:, c8, st * 128:(st + 1) * 128],
                                         wv3[:, c8, :], start=(c8 == 0), stop=(c8 == 7))
                    vt = pv.tile([128, 520], bf16, tag="vext", name=f"vext{st}")
                    v3 = vt[:].rearrange("p (h e) -> p h e", e=65)
                    nc.vector.tensor_copy(v3[:, :, 0:64], vps[:].rearrange("p (h e) -> p h e", e=64))
                    nc.vector.memset(v3[:, :, 64:65], 1.0)
                    vext[st] = vt

                # ---- Q^T / K^T projection pieces for head pair j ----
                qtb = [None] * 4
                ktb = [None] * 4

                def qk_alloc(j):
                    qtb[j] = pqk.tile([128, 1024], bf16, tag="qk", name=f"qt{j}")
                    ktb[j] = pqk.tile([128, 1024], bf16, tag="qk", name=f"kt{j}")

                _qk_ps = {}

                def qk_half(j, piece, half):
                    # piece 0..3: Q halves qc=0,1 then K halves qc=0,1;
                    # half 0/1 emits 4 of the 8 contraction matmuls so the
                    # PSUM chain can interleave with attention in ~850ns bites.
                    # Chains alternate between the pqs and pos pools so a
                    # piece's eviction latency never blocks the next piece.
                    qc = piece % 2
                    wsrc = wq3 if piece < 2 else wk3
                    key = (j, piece)
                    if half == 0:
                        pool_, ptag = (pqs, "qs") if piece % 2 == 0 else (pos, "os")
                        _qk_ps[key] = pool_.tile([128, 512], f32, tag=ptag,
                                                 name=f"qkps{j}_{piece}")
                    ps = _qk_ps[key]
                    for c8 in range(4 * half, 4 * half + 4):
                        nc.tensor.matmul(ps[:],
                                         wsrc[:, c8, j * 128:(j + 1) * 128],
                                         xt3[:, c8, qc * 512:(qc + 1) * 512],
                                         start=(c8 == 0), stop=(c8 == 7))
                    if half == 1:
                        if piece < 2:
                            nc.vector.tensor_scalar(
                                qtb[j][:, qc * 512:(qc + 1) * 512], ps[:],
                                scale_t[:, j:j + 1], bqs_t[:, j:j + 1],
                                ALU.mult, ALU.add)
                        else:
                            nc.vector.tensor_scalar_add(
                                ktb[j][:, qc * 512:(qc + 1) * 512], ps[:],
                                bk_t[:, j:j + 1])

                def qk_piece(j, piece):
                    qk_half(j, piece, 0)
                    qk_half(j, piece, 1)

                # ---- out projection chain for (qt, ch) ----
                attl = [None] * 4
                attl31 = [None]  # last window's half lives separately
                pending_norm = []
                pending_recip = []

                def outproj_open(qt, ch, pool_, ptag):
                    while pending_recip:
                        pending_recip.pop(0)()
                    while pending_norm:
                        pending_norm.pop(0)()
                    ops = pool_.tile([128, 512], f32, tag=ptag,
                                     name=f"ops{qt}_{ch}")
                    for rcx in range(3):
                        nc.tensor.matmul(ops[:],
                                         attl[rcx][:, qt * 128:(qt + 1) * 128],
                                         wo3[:, rcx, ch * 512:(ch + 1) * 512],
                                         start=(rcx == 0), stop=False)
                    return ops

                def outproj_close(qt, ch, ops):
                    if qt < 4:
                        st_ap = attl[3][:, qt * 128:(qt + 1) * 128]
                    else:
                        st_ap = attl31[0][:, (qt - 4) * 128:(qt - 3) * 128]
                    nc.tensor.matmul(ops[:], st_ap,
                                     wo3[:, 3, ch * 512:(ch + 1) * 512],
                                     start=False, stop=True)
                    ot = pout.tile([128, 512], f32, tag="out", name=f"ot{qt}_{ch}")
                    if ch == 0:
                        nc.vector.tensor_copy(ot[:], ops[:])
                    else:
                        nc.scalar.activation(ot[:], ops[:], AF.Copy)
                    nc.sync.dma_start(
                        out=out[qt * 128:(qt + 1) * 128, ch * 512:(ch + 1) * 512],
                        in_=ot[:])

                def outproj(qt, ch, pool_=None, ptag=None):
                    pool_ = pool_ or pos
                    outproj_close(qt, ch, outproj_open(qt, ch, pool_, ptag or "os"))

                # ---- attention half-window (j, qc): 512 q columns ----
                def attention_half(j, qc, filler):
                    # filler(kt) emits interleaved PE work after each kt's
                    # score matmuls so the tensor engine stays busy while ACT
                    # runs exp.
                    qs = slice(qc * 512, (qc + 1) * 512)
                    m0 = mh[2 * j][:].rearrange("p (k q) -> p k q", k=8)
                    m1 = mh[2 * j + 1][:].rearrange("p (k q) -> p k q", k=8)
                    avs = [pav.tile([65, 512], f32, tag="av", name=f"av{j}_{qc}_{a}")
                           for a in range(2)]
                    for kt in range(8):
                        if kt == 2:
                            while pending_recip:
                                pending_recip.pop(0)()
                        psA = psc.tile([128, 512], f32, tag="sc", name=f"sA{j}_{qc}_{kt}")
                        psB = psc.tile([128, 512], f32, tag="sc", name=f"sB{j}_{qc}_{kt}")
                        nc.tensor.matmul(psA[:],
                                         ktb[j][0:64, kt * 128:(kt + 1) * 128],
                                         qtb[j][0:64, qs],
                                         start=True, stop=True, tile_position=(0, 0))
                        nc.tensor.matmul(psB[:],
                                         ktb[j][64:128, kt * 128:(kt + 1) * 128],
                                         qtb[j][64:128, qs],
                                         start=True, stop=True, tile_position=(64, 0))
                        filler(kt)
                        for a, sps, mv in ((0, psA, m0), (1, psB, m1)):
                            pt = ppt.tile([128, 512], bf16, tag="pt",
                                          name=f"pt{j}_{qc}_{a}_{kt}")
                            nc.scalar.activation(pt[:], sps[:], AF.Exp)
                            nc.vector.tensor_mul(pt[:], pt[:], mv[:, kt, qs])
                            hh = 2 * j + a
                            nc.tensor.matmul(avs[a][0:65, :],
                                             vext[kt][:, hh * 65:(hh + 1) * 65],
                                             pt[:],
                                             start=(kt == 0), stop=(kt == 7))

                    while pending_norm:
                        pending_norm.pop(0)()
                    # normalize, fully decoupled from window boundaries:
                    # the recip+broadcast chain of this window is deferred to
                    # the next window's interior (kt==2); the avs->attl
                    # multiplies run a window later still, when the broadcast
                    # has long landed.
                    if attl[j] is None:
                        attl[j] = pattl.tile([128, 1024], bf16, tag="attl",
                                             name=f"attl{j}")
                    last = (j == 3 and qc == 1)
                    if last:
                        attl31[0] = pattl.tile([128, 512], bf16, tag="attl31",
                                               name="attl31")
                    rc = pdm.tile([1, 1024], f32, tag="rc", name=f"rc{j}_{qc}")
                    rb = prb.tile([128, 512], f32, tag="rb", name=f"rb{j}_{qc}")
                    rcd = pdram.tile([1, 1024], f32, tag="rcd", name=f"rcd{j}_{qc}")

                    def recip_chain(avs=avs, rc=rc, rb=rb, rcd=rcd):
                        for a in range(2):
                            nc.vector.reciprocal(rc[0:1, a * 512:(a + 1) * 512],
                                                 avs[a][64:65, :])
                        nc.sync.dma_start(out=rcd[:], in_=rc[:])
                        for a in range(2):
                            nc.sync.dma_start(
                                out=rb[a * 64:(a + 1) * 64, :],
                                in_=rcd[0:1, a * 512:(a + 1) * 512]
                                .to_broadcast((64, 512)))

                    if last:
                        at_, qs_ = attl31[0], slice(0, 512)
                        recip_chain()
                    else:
                        at_, qs_ = attl[j], qs
                        pending_recip.append(recip_chain)

                    def norm_muls(at=at_, avs=avs, rb=rb, qs=qs_):
                        for a in range(2):
                            nc.vector.tensor_mul(
                                at[a * 64:(a + 1) * 64, qs],
                                avs[a][0:64, :],
                                rb[a * 64:(a + 1) * 64, :])
                    pending_norm.append(norm_muls)

                # ---- schedule: filler lists give each kt slot ~850ns of
                # independent PE work so the tensor engine never starves while
                # ACT runs exp ----
                qk_alloc(0)
                for piece in range(4):
                    qk_piece(0, piece)
                vchunk(0)
                vchunk(1)

                def F(*items):
                    def f(kt):
                        if kt < len(items) and items[kt] is not None:
                            items[kt]()
                    return f

                def vch(st):
                    return lambda: vchunk(st)

                def qkh(j, piece, half):
                    def g():
                        if j is not None and piece == 0 and half == 0:
                            qk_alloc(j)
                        qk_half(j, piece, half)
                    return g

                def op(qt, ch):
                    return lambda: outproj(qt, ch)

                load_mask(2)
                attention_half(0, 0, F(vch(2), vch(3), vch(4), vch(5), vch(6),
                                       vch(7), qkh(1, 0, 0), qkh(1, 0, 1)))
                load_mask(3)
                attention_half(0, 1, F(qkh(1, 1, 0), qkh(1, 1, 1),
                                       qkh(1, 2, 0), None, qkh(1, 2, 1), None,
                                       None, None))
                load_mask(4)
                attention_half(1, 0, F(qkh(1, 3, 0), qkh(1, 3, 1),
                                       qkh(2, 0, 0), None, qkh(2, 0, 1), None,
                                       None, None))
                load_mask(5)
                nc.sync.dma_start(out=wob[:].rearrange("p (r n) -> p r n", r=4),
                                  in_=wo[:].rearrange("(r p) n -> p r n", p=128))
                attention_half(1, 1, F(qkh(2, 1, 0), None, qkh(2, 1, 1), None,
                                       qkh(2, 2, 0), None, qkh(2, 2, 1), None))
                load_mask(6)
                attention_half(2, 0, F(qkh(2, 3, 0), qkh(2, 3, 1),
                                       qkh(3, 0, 0), None, qkh(3, 0, 1), None,
                                       None, None))
                load_mask(7)
                attention_half(2, 1, F(qkh(3, 1, 0), None, qkh(3, 1, 1), None,
                                       qkh(3, 2, 0), None, qkh(3, 2, 1), None))
                attention_half(3, 0, F(qkh(3, 3, 0), qkh(3, 3, 1)))
                def opr(qt, ch, pool_, ptag):
                    return lambda: outproj(qt, ch, pool_, ptag)

                attention_half(3, 1, F(
                    None, None,
                    opr(0, 0, pos, "os"), opr(0, 1, pqs, "qs"),
                    opr(1, 0, pos, "os"), opr(1, 1, pqs, "qs"),
                    opr(2, 0, pos, "os"), opr(2, 1, pqs, "qs")))
                outproj(3, 0, pos, "os")
                outproj(3, 1, pqs, "qs")
                tail_rot = [(pos, "os"), (pqs, "qs"), (psc, "sc"), (psc, "sc")]
                for grp in ([(4, 0), (4, 1), (5, 0), (5, 1)],
                            [(6, 0), (6, 1), (7, 0), (7, 1)]):
                    opens = []
                    for i, (qt, ch) in enumerate(grp):
                        pool_, ptag = tail_rot[i]
                        opens.append((qt, ch, outproj_open(qt, ch, pool_, ptag)))
                    for qt, ch, ops in opens:
                        outproj_close(qt, ch, ops)

    nc.compile()
    return nc


def _get_nc():
    global _NC_CACHE
    if _NC_CACHE is None:
        _NC_CACHE = _build()
    return _NC_CACHE


def _prep_inputs(x, Wq, bq, Wk, bk, Wv, bv, Wo, bo, temperature, sparse_mask):
    bfd = ml_dtypes.bfloat16
    x = np.asarray(x, np.float32)
    Wq = np.asarray(Wq, np.float32); Wk = np.asarray(Wk, np.float32)
    Wv = np.asarray(Wv, np.float32); Wo = np.asarray(Wo, np.float32)
    bq = np.asarray(bq, np.float32); bk = np.asarray(bk, np.float32)
    temp = np.asarray(temperature, np.float32).reshape(-1)
    mask = np.asarray(sparse_mask)

    in_maps = []
    for c in CORE_IDS:
        b, g = c // 2, c % 2
        cols = slice(g * LOC, (g + 1) * LOC)
        hs = slice(g * GH, (g + 1) * GH)
        in_maps.append({
            "xT": np.ascontiguousarray(x[b].T).astype(bfd),
            "wq": np.ascontiguousarray(Wq[:, cols]).astype(bfd),
            "wk": np.ascontiguousarray(Wk[:, cols]).astype(bfd),
            "wv": np.ascontiguousarray(Wv[:, cols]).astype(bfd),
            "wo": np.ascontiguousarray(Wo[cols, :]).astype(bfd),
            "maskT": np.ascontiguousarray(
                mask[b, hs].transpose(0, 2, 1)).astype(bfd),
            "tempx": (np.repeat(temp[hs], D) / np.sqrt(D)).astype(np.float32),
            "bqv": np.ascontiguousarray(bq[cols]),
            "bkv": np.ascontiguousarray(bk[cols]),
        })
    return in_maps


def kernel(**inputs):
    in_maps = _prep_inputs(**inputs)
    nc = _get_nc()
    res = run_bass_kernel_spmd(nc, in_maps, CORE_IDS)
    # unshard: row-parallel partial sum per batch + constant bias row
    # (softmax rows sum to 1 so bv contributes bv @ Wo to every row)
    bv = np.asarray(inputs["bv"], np.float32)
    bo = np.asarray(inputs["bo"], np.float32)
    Wo = np.asarray(inputs["Wo"], np.float32)
    brow = bv @ Wo + bo
    out = np.empty((B, S, HID), np.float32)
    for b in range(B):
        out[b] = res.results[2 * b]["out"] + res.results[2 * b + 1]["out"] + brow
    return out
